# revision 22
# baseline (speedup 1.0000x reference)
"""DGCNN encoder Bass kernel for 8 Trainium2 NeuronCores.

Hardcoded for B=2,T=2,N=256,K=27. Channel-split of the conv layers across
8 cores (L0 replicated); knn on-device (DVE top-8 rounds, f32-exact);
neighbor gather as one-hot matmuls on TensorE in fp16 (single precision,
no hi/lo split); adjacency-count matrix A via gpsimd local_scatter +
PE transpose; gather-free BatchNorm stats via count matmuls; max-over-K
split between DVE grouped reduce and Act-copy + DVE fp16 in-place tree;
BN+LeakyReLU fused into one Activation (Prelu) op; fp16 AllGathers +
one f32 AllReduce.
"""
import sys
sys.path.insert(0, "/opt/trn_rl_repo")
import numpy as np

import concourse.bass as bass
import concourse.bacc as bacc
import concourse.mybir as mybir
import concourse.tile as tile
from concourse.bass_utils import run_bass_kernel_spmd

F32 = mybir.dt.float32
F16 = mybir.dt.float16
I16 = mybir.dt.int16
U16 = mybir.dt.uint16
AX = mybir.AxisListType.X
OP = mybir.AluOpType
AF = mybir.ActivationFunctionType

NCORE = 8
B, T, N, K = 2, 2, 256, 27
BT = B * T
PTS = BT * N          # 1024
NK = N * K            # 6912
QK = NK // 4          # 1728 cols = one chunk-pair
EPS = 1e-5
COUT = [48, 192, 768, 3072]
FIN = [3, 96, 384, 1536]
SL = [48, 24, 96, 384]        # per-core out-channels (L0 replicated=full)
NCT = [1, 1, 1, 3]
MBN = BT * N * K

# per (f, ct): 4 chunk-pairs; "B" = Act copy + DVE fp16 tree (2 chunks
# fused), "A" = 2x DVE grouped reduce direct from PSUM.
ROUTE = ("B", "B", "B", "A")


import os
STAGE = int(os.environ.get("DGCNN_STAGE", "9"))
SUB = int(os.environ.get("DGCNN_SUB", "9"))


def _cdiv(a, b):
    return (a + b - 1) // b


def build_nc():
    nc = bacc.Bacc("TRN2", target_bir_lowering=False, debug=False,
                   num_devices=NCORE)
    dt = nc.dram_tensor
    io = {}
    io["x16"] = dt("x16", [3, PTS], F16, kind="ExternalInput")
    io["xf32"] = dt("xf32", [3, PTS], F32, kind="ExternalInput")
    io["iota"] = dt("iota", [256, 1], F32, kind="ExternalInput")
    io["negones"] = dt("negones", [3, 128], F32, kind="ExternalInput")
    for li in range(4):
        f = FIN[li]
        io[f"wa{li}"] = dt(f"wa{li}", [f, SL[li]], F16, kind="ExternalInput")
        io[f"wd{li}"] = dt(f"wd{li}", [f, SL[li]], F16, kind="ExternalInput")
        io[f"g{li}"] = dt(f"g{li}", [SL[li], 1], F32, kind="ExternalInput")
        io[f"b{li}"] = dt(f"b{li}", [SL[li], 1], F32, kind="ExternalInput")
    io["w4t"] = dt("w4t", [1104, 256], F16, kind="ExternalInput")
    io["g4"] = dt("g4", [256, 1], F32, kind="ExternalInput")
    io["b4"] = dt("b4", [256, 1], F32, kind="ExternalInput")
    out_d = dt("out", [B, T, 256, N], F32, kind="ExternalOutput")

    flatidx = dt("flatidx", [BT, NK], F16)
    ag1_in = dt("ag1_in", [2 * SL[1], PTS], F16)
    ag1_out = dt("ag1_out", [2 * COUT[1], PTS], F16, addr_space="Shared")
    ag2_in = dt("ag2_in", [2 * SL[2], PTS], F16)
    ag2_out = dt("ag2_out", [2 * COUT[2], PTS], F16, addr_space="Shared")
    ar_in = dt("ar_in", [256, PTS], F32)
    ar_out = dt("ar_out", [256, PTS], F32, addr_space="Shared")
    DEBUG = os.environ.get("DGCNN_DEBUG", "") == "1"
    if DEBUG:
        dbg = {}
        dbg["u0"] = dt("dbg_u0", [128, 48], F16, kind="ExternalOutput")
        dbg["v0"] = dt("dbg_v0", [48, PTS], F16, kind="ExternalOutput")
        dbg["up0"] = dt("dbg_up0", [48, PTS], F16, kind="ExternalOutput")
        dbg["hp0"] = dt("dbg_hp0", [48, PTS], F16, kind="ExternalOutput")
        dbg["cb"] = dt("dbg_cb", [128, 64], F32, kind="ExternalOutput")
        dbg["a00"] = dt("dbg_a00", [128, 256], F16, kind="ExternalOutput")
        dbg["c00"] = dt("dbg_c00", [128, 1], F16, kind="ExternalOutput")
        dbg["hp1"] = dt("dbg_hp1", [SL[1], PTS], F16, kind="ExternalOutput")
        dbg["hp2"] = dt("dbg_hp2", [SL[2], PTS], F16, kind="ExternalOutput")
        for c in range(3):
            dbg[f"hp3_{c}"] = dt(f"dbg_hp3_{c}", [128, PTS], F16,
                                 kind="ExternalOutput")
        for c in range(3):
            dbg[f"h16_{c}"] = dt(f"dbg_h16_{c}", [128, PTS], F16,
                                 kind="ExternalOutput")
        dbg["sb3"] = dt("dbg_sb3", [128, 8], F32, kind="ExternalOutput")
        for pt in range(8):
            dbg[f"u3_{pt}"] = dt(f"dbg_u3_{pt}", [128, 384], F16,
                                 kind="ExternalOutput")
        dbg["cnt"] = dt("dbg_cnt", [128, 8], F32, kind="ExternalOutput")
        for c in range(3):
            dbg[f"up3_{c}"] = dt(f"dbg_up3_{c}", [128, PTS], F16,
                                 kind="ExternalOutput")
            dbg[f"v3_{c}"] = dt(f"dbg_v3_{c}", [128, PTS], F16,
                                kind="ExternalOutput")
        dbg["cb1"] = dt("dbg_cb1", [128, 64], F32, kind="ExternalOutput")
        dbg["cb2"] = dt("dbg_cb2", [128, 64], F32, kind="ExternalOutput")
        dbg["cb3"] = dt("dbg_cb3", [128, 64], F32, kind="ExternalOutput")
    rg = [list(range(NCORE))]

    with tile.TileContext(nc) as tc:
        pers = tc.alloc_tile_pool(name="pers", bufs=1)
        work = tc.alloc_tile_pool(name="work", bufs=2)
        p1 = tc.alloc_tile_pool(name="p1", bufs=3, space="PSUM")
        pa = tc.alloc_tile_pool(name="pa", bufs=2, space="PSUM")

        v = nc.vector
        sc = nc.scalar
        te = nc.tensor
        gp = nc.gpsimd
        sy = nc.sync

        # ---------------- constants ----------------
        iota_f = []
        for t in range(2):
            it_f = pers.tile([128, 1], F32, tag=f"iotaf{t}", name=f"iotaf{t}")
            sy.dma_start(it_f[:], io["iota"].ap()[t * 128:(t + 1) * 128])
            iota_f.append(it_f)
        negones = pers.tile([3, 128], F32, tag="negones", name="negones")
        sy.dma_start(negones[:], io["negones"].ap())
        eps_col = pers.tile([128, 1], F32, tag="eps_col", name="eps_col")
        v.memset(eps_col[:], EPS)
        # xf shares the hwork tag (only needed during knn, long before L4)
        xf = work.tile([3, PTS], F32, tag="hwork", name="xf")
        sy.dma_start(xf[:], io["xf32"].ap())
        x16 = pers.tile([3, PTS], F16, tag="x16", name="x16")
        sy.dma_start(x16[:], io["x16"].ap())
        ones28 = pers.tile([128, 28], F16, tag="ones28", name="ones28")
        v.memset(ones28[:], 1.0)
        # identity (fp16) for PE transposes
        irow = pers.tile([128, 128], F32, tag="irow", name="irow")
        sy.dma_start(irow[:],
                     io["iota"].ap().rearrange("n 1 -> 1 n")[0:1, 0:128]
                     .broadcast_to([128, 128]))
        ident = pers.tile([128, 128], F16, tag="ident", name="ident")
        v.tensor_scalar(ident[:], irow[:], iota_f[0][:], None, OP.is_equal)
        # per-layer gamma/beta columns
        gcols, bcols = [], []
        for li in range(4):
            gs, bs = [], []
            for ct in range(NCT[li]):
                cw = min(128, SL[li] - ct * 128)
                g_ = pers.tile([cw, 1], F32, tag=f"g{li}_{ct}",
                               name=f"g{li}_{ct}")
                sy.dma_start(g_[:], io[f"g{li}"].ap()[ct * 128:ct * 128 + cw])
                b_ = pers.tile([cw, 1], F32, tag=f"b{li}_{ct}",
                               name=f"b{li}_{ct}")
                sy.dma_start(b_[:], io[f"b{li}"].ap()[ct * 128:ct * 128 + cw])
                gs.append(g_)
                bs.append(b_)
            gcols.append(gs)
            bcols.append(bs)
        # scratch column bank (f32): per-ct stat/finalize columns
        colbank = pers.tile([128, 64], F32, tag="colbank", name="colbank")
        dbg_sb = pers.tile([128, 8], F32, tag="dbg_sb", name="dbg_sb")

        # persistent per-frame structures
        G = [[pers.tile([128, NK], F16, tag=f"G{f}_{t}", name=f"G{f}_{t}")
              for t in range(2)] for f in range(BT)]
        A = [[pers.tile([128, N], F16, tag=f"A{f}_{t}", name=f"A{f}_{t}")
              for t in range(2)] for f in range(BT)]
        CNT16 = [[pers.tile([128, 1], F16, tag=f"c16_{f}_{t}",
                            name=f"c16_{f}_{t}") for t in range(2)]
                 for f in range(BT)]
        CNT32 = [[pers.tile([128, 1], F32, tag=f"c32_{f}_{t}",
                            name=f"c32_{f}_{t}") for t in range(2)]
                 for f in range(BT)]

        # resident activations (local combs, fp16)
        hp0 = pers.tile([48, PTS], F16, tag="hp0", name="hp0")
        tp0 = pers.tile([48, PTS], F16, tag="tp0", name="tp0")
        hp1 = pers.tile([SL[1], PTS], F16, tag="hp1", name="hp1")
        tp1r = pers.tile([SL[1], 512], F16, tag="tp1r", name="tp1r")
        hp2 = pers.tile([SL[2], PTS], F16, tag="hp2", name="hp2")
        tp2r = pers.tile([SL[2], 512], F16, tag="tp2r", name="tp2r")
        hp3 = [pers.tile([128, PTS], F16, tag=f"hp3_{ct}", name=f"hp3_{ct}")
               for ct in range(3)]
        tp3r = [pers.tile([128, 512], F16, tag=f"tp3r_{ct}",
                          name=f"tp3r_{ct}") for ct in range(3)]
        V = [pers.tile([128, PTS], F16, tag=f"V{ct}", name=f"V{ct}")
             for ct in range(3)]
        upool = [pers.tile([128, PTS], F16, tag=f"up{ct}", name=f"up{ct}")
                 for ct in range(3)]

        # ---------------- knn + A ----------------
        for f in range(BT):
            xt = xf[:, f * N:(f + 1) * N]
            xsqh = work.tile([3, N], F32, tag="xsqh", name="xsqh", bufs=1)
            v.scalar_tensor_tensor(xsqh[:], xt, 0.5, xt, OP.mult, OP.mult)
            for t in range(2):
                ps = pa.tile([128, 512], F32, tag="pa", name="knn_ps")[:, :N]
                te.matmul(ps, xt[:, t * 128:(t + 1) * 128], xt,
                          start=True, stop=False)
                te.matmul(ps, negones[:], xsqh[:], start=False, stop=True)
                s_a = work.tile([128, N], F32, tag="s_a", name="s_a", bufs=1)
                sc.copy(s_a[:], ps)
                s_b = work.tile([128, N], F32, tag="s_b", name="s_b",
                                bufs=1)
                idx32 = work.tile([128, 32], U16, tag="idx32", name="idx32", bufs=1)
                cur, nxt = s_a, s_b
                for r in range(4):
                    m8 = work.tile([128, 8], F32, tag="m8", name="m8", bufs=1)
                    v.max(m8[:], cur[:])
                    v.max_index(idx32[:, 8 * r:8 * r + 8], m8[:], cur[:])
                    if r < 3:
                        v.match_replace(nxt[:], m8[:], cur[:], -3.0e38)
                        cur, nxt = nxt, cur
                idxf = work.tile([128, K], F32, tag="idxf", name="idxf", bufs=1)
                v.tensor_copy(idxf[:], idx32[:, :K])
                idxb = work.tile([128, K], F16, tag="idxb", name="idxb", bufs=1)
                sc.copy(idxb[:], idxf[:])
                dst = flatidx.ap()[f].rearrange("(n k) -> n k", k=K)
                sy.dma_start(dst[t * 128:(t + 1) * 128], idxb[:])
                # A^T rows for this point-tile via local scatter
                idx28 = work.tile([128, 28], I16, tag="idx28", name="idx28", bufs=1)
                v.memset(idx28[:], -1)
                v.tensor_copy(idx28[:, 0:K], idxf[:])
                at = work.tile([128, N], F16, tag="at", name="at", bufs=1)
                gp.local_scatter(at[:], ones28[:], idx28[:], 128, N, 28)
                for mt in range(2):
                    tpp = pa.tile([128, 1024], F16, tag="pa",
                                  name="tr_ps")[:, :128]
                    te.transpose(tpp, at[:, mt * 128:(mt + 1) * 128],
                                 ident[:])
                    sc.copy(A[f][mt][:, t * 128:(t + 1) * 128], tpp)
            for mt in range(2):
                with nc.allow_low_precision(reason="exact int counts"):
                    v.tensor_reduce(CNT16[f][mt][:], A[f][mt][:], axis=AX,
                                    op=OP.add)
                sc.copy(CNT32[f][mt][:], CNT16[f][mt][:])

        # ---------------- G build (one-hot, fp16, resident) ----------------
        HK = NK // 8
        for f in range(BT):
            for q in range(8):
                bc = work.tile([128, HK], F16, tag="bc", name="bc", bufs=1)
                sy.dma_start(
                    bc[:],
                    flatidx.ap()[f:f + 1, q * HK:(q + 1) * HK]
                    .broadcast_to([128, HK]))
                for t in range(2):
                    v.tensor_scalar(G[f][t][:, q * HK:(q + 1) * HK], bc[:],
                                    iota_f[t][:], None, OP.is_equal)

        # ---------------- layers ----------------
        U = [None] * 8

        for li in range(min(4, STAGE)):
            C8 = SL[li]
            nct = NCT[li]
            # fblocks (fp16): list of (tile_ap, rows)
            if li == 0:
                fbs = [(x16, 3)]
            elif li == 1:
                fbs = [(hp0, 48), (tp0, 48)]
            else:
                ag = ag1_out if li == 2 else ag2_out
                F = FIN[li]
                fbs = []
                for kb in range(F // 128):
                    fb = work.tile([128, PTS], F16, tag=f"fb{kb}",
                                   name=f"fb{kb}", bufs=1)
                    sy.dma_start(fb[:], ag.ap()[kb * 128:(kb + 1) * 128])
                    fbs.append((fb, 128))

            # ---- U build: 2 rounds of 4 pt ----
            # rs2 (sum cnt*U^2) accumulates in one isolated PSUM bank per
            # ct (bank 1 of pxa/pxb/pxc) -- start=True zeroes a whole
            # bank, so no two accumulation groups may share one.
            pxa = p1.tile([128, 864], F32, tag="px1", name="pxa")
            pxb = p1.tile([128, 864], F32, tag="px1", name="pxb")
            pxc = p1.tile([128, 864], F32, tag="px1", name="pxc")
            pau = pa.tile([128, 512], F32, tag="pa", name="pau")
            pxl = [pxa, pxb, pxc]
            gslot = [pxa[:, 0:C8], pxb[:, 0:C8], pxc[:, 0:C8],
                     pau[:, 0:C8]]
            for rnd in range(2 if SUB >= 1 else 0):
                ro = 0
                for bi, (fb, rows) in enumerate(fbs):
                    wa_b = work.tile([128, 384], F16, tag="wstream",
                                     name="wa_b", bufs=3)[:rows, :C8]
                    sy.dma_start(wa_b[:], io[f"wa{li}"].ap()[ro:ro + rows])
                    for j in range(4):
                        pt = rnd * 4 + j
                        te.matmul(gslot[j],
                                  fb[0:rows, pt * 128:(pt + 1) * 128],
                                  wa_b[:], start=(bi == 0),
                                  stop=(bi == len(fbs) - 1))
                    ro += rows
                for j in range(4):
                    pt = rnd * 4 + j
                    u_ = pers.tile([128, 384], F16, tag=f"u{pt}",
                                   name=f"u{pt}")[:, :C8]
                    sc.copy(u_[:], gslot[j])
                    U[pt] = u_
                    sqf = work.tile([128, 384], F32, tag="scr",
                                    name="sqf", bufs=1)[:, :C8]
                    sc.square(sqf[:], u_[:])
                    f_, t_ = pt // 2, pt % 2
                    for ct in range(nct):
                        cw = min(128, C8 - ct * 128)
                        cs = ct * 128
                        te.matmul(pxl[ct][:cw, 512:513],
                                  sqf[:, cs:cs + cw], CNT32[f_][t_][:],
                                  start=(pt == 0), stop=(pt == 7))
            # fold rs2 -> colbank col 16+ct
            for ct in range(nct if SUB >= 1 else 0):
                cw = min(128, C8 - ct * 128)
                v.tensor_copy(colbank[:cw, 16 + ct:17 + ct],
                              pxl[ct][:cw, 512:513])

            # ---- V build (+ V stats via Act accum) ----
            # colbank cols: rv halves 20+2ct, rv2 halves 26+2ct
            for ct in range(nct if SUB >= 2 else 0):
                cw = min(128, C8 - ct * 128)
                cs = ct * 128
                pv0 = pa.tile([128, 512], F32, tag="pa", name="pv0")[:cw, :]
                pv1 = pa.tile([128, 512], F32, tag="pa", name="pv1")[:cw, :]
                ro = 0
                for bi, (fb, rows) in enumerate(fbs):
                    wd_b = work.tile([128, 384], F16, tag="wstream",
                                     name="wd_b", bufs=3)[:rows, :cw]
                    sy.dma_start(wd_b[:],
                                 io[f"wd{li}"].ap()[ro:ro + rows,
                                                    cs:cs + cw])
                    te.matmul(pv0[:], wd_b[:], fb[0:rows, 0:512],
                              start=(bi == 0), stop=(bi == len(fbs) - 1))
                    te.matmul(pv1[:], wd_b[:], fb[0:rows, 512:1024],
                              start=(bi == 0), stop=(bi == len(fbs) - 1))
                    ro += rows
                sc.activation(V[ct][:cw, 0:512], pv0[:], AF.Copy,
                              accum_out=colbank[:cw, 20 + 2 * ct:
                                                21 + 2 * ct])
                sc.activation(V[ct][:cw, 512:1024], pv1[:], AF.Copy,
                              accum_out=colbank[:cw, 21 + 2 * ct:
                                                22 + 2 * ct])
                tr0 = work.tile([128, 768], F16, tag="scr", name="tr0",
                                bufs=1)[:cw, :512]
                sc.activation(tr0[:], pv0[:], AF.Square,
                              accum_out=colbank[:cw, 26 + 2 * ct:
                                                27 + 2 * ct])
                tr1 = work.tile([128, 768], F16, tag="scr", name="tr1",
                                bufs=1)[:cw, :512]
                sc.activation(tr1[:], pv1[:], AF.Square,
                              accum_out=colbank[:cw, 27 + 2 * ct:
                                                28 + 2 * ct])

            # ---- expansion: per frame, per ct, 4 chunk-pairs ----
            for f in range(BT if SUB >= 3 else 0):
                for ct in range(nct):
                    cw = min(128, C8 - ct * 128)
                    cs = ct * 128
                    for pr in range(4):
                        base = pr * QK
                        up_dst = upool[ct][:cw,
                                           f * N + pr * 64:f * N + pr * 64
                                           + 64]
                        if ROUTE[pr] == "B":
                            e2 = work.tile([128, QK], F16, tag="E2",
                                           name="e2")[:cw, :]
                            for half in range(2):
                                px = p1.tile([128, 864], F32, tag="px1",
                                             name="px1")[:cw, :]
                                hb = base + half * 864
                                for lo, hi in ((0, 512), (512, 864)):
                                    for t in range(2):
                                        te.matmul(
                                            px[:, lo:hi],
                                            U[2 * f + t][:, cs:cs + cw],
                                            G[f][t][:, hb + lo:hb + hi],
                                            start=(t == 0), stop=(t == 1))
                                sc.copy(e2[:, half * 864:(half + 1) * 864],
                                        px[:])
                            # in-place fp16 max tree over k=27 (64 groups)
                            e3 = e2.rearrange("p (g k) -> p g k", k=K)
                            v.tensor_tensor(e3[:, :, 0:13], e3[:, :, 0:13],
                                            e3[:, :, 13:26], OP.max)
                            v.tensor_tensor(e3[:, :, 0:6], e3[:, :, 0:6],
                                            e3[:, :, 6:12], OP.max)
                            v.tensor_tensor(e3[:, :, 0:3], e3[:, :, 0:3],
                                            e3[:, :, 3:6], OP.max)
                            v.tensor_tensor(e3[:, :, 0:1], e3[:, :, 0:1],
                                            e3[:, :, 1:2], OP.max)
                            v.tensor_tensor(e3[:, :, 0:1], e3[:, :, 0:1],
                                            e3[:, :, 2:3], OP.max)
                            v.tensor_tensor(e3[:, :, 0:1], e3[:, :, 0:1],
                                            e3[:, :, 12:13], OP.max)
                            v.tensor_tensor(
                                up_dst.rearrange("p (g k) -> p g k", k=1),
                                e3[:, :, 0:1], e3[:, :, 26:27], OP.max)
                        else:
                            for half in range(2):
                                px = p1.tile([128, 864], F32, tag="px1",
                                             name="px1")[:cw, :]
                                hb = base + half * 864
                                for lo, hi in ((0, 512), (512, 864)):
                                    for t in range(2):
                                        te.matmul(
                                            px[:, lo:hi],
                                            U[2 * f + t][:, cs:cs + cw],
                                            G[f][t][:, hb + lo:hb + hi],
                                            start=(t == 0), stop=(t == 1))
                                v.tensor_reduce(
                                    up_dst[:, half * 32:half * 32 + 32],
                                    px[:].rearrange("p (g k) -> p g k",
                                                    k=K),
                                    axis=AX, op=OP.max)
                    # cross-term: rvs col = sum_n V * (U @ A)
                    if SUB < 4:
                        continue
                    ps1 = pa.tile([128, 512], F32, tag="pa",
                                  name="ps1")[:cw, :N]
                    for t in range(2):
                        te.matmul(ps1, U[2 * f + t][:, cs:cs + cw],
                                  A[f][t][:], start=(t == 0), stop=(t == 1))
                    tto = work.tile([128, N], F16, tag="ttro", name="tto",
                                    bufs=1)[:cw, :]
                    v.scalar_tensor_tensor(
                        tto[:], V[ct][:cw, f * N:(f + 1) * N], 1.0, ps1,
                        OP.mult, OP.mult,
                        accum_out=colbank[:cw, 4 * ct + f:4 * ct + f + 1])
                    v.tensor_reduce(
                        colbank[:cw, 32 + 4 * ct + f:33 + 4 * ct + f],
                        ps1, axis=AX, op=OP.add)

            # ---- finalize per ct: BN cols, fused BN+lrelu, tp ----
            for ct in range(nct if SUB >= 5 else 0):
                cw = min(128, C8 - ct * 128)
                cb = colbank[:cw, :]
                col = lambda i: cb[:, i:i + 1]
                base = 44 + 6 * ct
                # rs = sum over frames of ps1 sums
                v.tensor_reduce(col(12 + ct),
                                cb[:, 32 + 4 * ct:32 + 4 * ct + 4],
                                axis=AX, op=OP.add)
                # rvs_sum over frames
                v.tensor_reduce(col(base + 0),
                                cb[:, 4 * ct:4 * ct + 4], axis=AX,
                                op=OP.add)
                v.tensor_tensor(col(base + 1), col(20 + 2 * ct),
                                col(21 + 2 * ct), OP.add)   # rv
                v.tensor_tensor(col(base + 2), col(26 + 2 * ct),
                                col(27 + 2 * ct), OP.add)   # rv2
                v.scalar_tensor_tensor(col(base + 3), col(base + 1),
                                       float(K), col(12 + ct), OP.mult,
                                       OP.add)              # sum_h
                v.scalar_tensor_tensor(col(base + 4), col(base + 0), 2.0,
                                       col(16 + ct), OP.mult, OP.add)
                v.scalar_tensor_tensor(col(base + 4), col(base + 2),
                                       float(K), col(base + 4), OP.mult,
                                       OP.add)               # sum_h2
                v.tensor_scalar(col(base + 3), col(base + 3), 1.0 / MBN,
                                None, OP.mult)               # mean
                v.tensor_scalar(col(base + 4), col(base + 4), 1.0 / MBN,
                                None, OP.mult)               # ex2
                v.tensor_tensor(col(base + 5), col(base + 3), col(base + 3),
                                OP.mult)
                v.tensor_tensor(col(base + 5), col(base + 4), col(base + 5),
                                OP.subtract)                 # var
                std = work.tile([128, 1], F32, tag="stdcol", name="std")
                sc.activation(std[:cw, :], col(base + 5), AF.Sqrt,
                              bias=eps_col[:cw, :])
                inv = work.tile([128, 1], F32, tag="invcol", name="inv")
                v.reciprocal(inv[:cw, :], std[:cw, :])
                scal = work.tile([128, 1], F32, tag="scalcol", name="scal")
                v.tensor_tensor(scal[:cw, :], inv[:cw, :],
                                gcols[li][ct][:], OP.mult)
                bias2 = work.tile([128, 1], F32, tag="biascol", name="bias2")
                v.tensor_tensor(bias2[:cw, :], col(base + 3), scal[:cw, :],
                                OP.mult)
                v.tensor_tensor(bias2[:cw, :], bcols[li][ct][:],
                                bias2[:cw, :], OP.subtract)
                # h = upool + V (fp16), then fused BN+lrelu on Act
                h16 = work.tile([128, PTS], F16, tag="h16", name="h16")
                v.tensor_tensor(h16[:cw, :], upool[ct][:cw, :],
                                V[ct][:cw, :], OP.add)
                if DEBUG and li == 3:
                    sy.dma_start(dbg[f"h16_{ct}"].ap(), h16[:])
                    v.tensor_copy(dbg_sb[:cw, 2 * ct:2 * ct + 1],
                                  scal[:cw, :])
                    v.tensor_copy(dbg_sb[:cw, 2 * ct + 1:2 * ct + 2],
                                  bias2[:cw, :])
                if li == 0:
                    hdst = hp0[:, :]
                elif li == 1:
                    hdst = hp1[:, :]
                elif li == 2:
                    hdst = hp2[:, :]
                else:
                    hdst = hp3[ct][:, :]
                sc.activation(hdst, h16[:cw, :], AF.Prelu,
                              bias=bias2[:cw, :], scale=scal[:cw, :],
                              alpha=0.2)
                # temporal pool: tp = h[f0] + h[f1] (x0.5 folded into
                # downstream weights); replicate across frames
                if li == 0:
                    v.tensor_tensor(tp0[:, 0:N], hdst[:, 0:N],
                                    hdst[:, N:2 * N], OP.add)
                    for rep in range(1, 4):
                        sc.copy(tp0[:, rep * N:(rep + 1) * N],
                                tp0[:, 0:N])
                else:
                    tdst = (tp1r, tp2r, None)[li - 1]
                    tdst = tp3r[ct] if li == 3 else tdst
                    v.tensor_tensor(tdst[:cw, 0:N], hdst[:, 0:N],
                                    hdst[:, N:2 * N], OP.add)
                    sc.copy(tdst[:cw, N:2 * N], tdst[:cw, 0:N])
                if li == 1:
                    sy.dma_start(ag1_in.ap()[0:SL[1]], hp1[:])
                    sy.dma_start(ag1_in.ap()[SL[1]:2 * SL[1], 0:512],
                                 tp1r[:])
                    sy.dma_start(ag1_in.ap()[SL[1]:2 * SL[1], 512:1024],
                                 tp1r[:])
                elif li == 2:
                    sy.dma_start(ag2_in.ap()[0:SL[2]], hp2[:])
                    sy.dma_start(ag2_in.ap()[SL[2]:2 * SL[2], 0:512],
                                 tp2r[:])
                    sy.dma_start(ag2_in.ap()[SL[2]:2 * SL[2], 512:1024],
                                 tp2r[:])
            if DEBUG and li == 0:
                sy.dma_start(dbg["u0"].ap(), U[0][:])
                sy.dma_start(dbg["v0"].ap(), V[0][:48, :])
                sy.dma_start(dbg["up0"].ap(), upool[0][:48, :])
                sy.dma_start(dbg["hp0"].ap(), hp0[:])
                sy.dma_start(dbg["cb"].ap(), colbank[:])
                sy.dma_start(dbg["a00"].ap(), A[0][0][:])
                sy.dma_start(dbg["c00"].ap(), CNT16[0][0][:])
            if DEBUG and li == 1:
                sy.dma_start(dbg["hp1"].ap(), hp1[:])
                sy.dma_start(dbg["cb1"].ap(), colbank[:])
            if DEBUG and li == 2:
                sy.dma_start(dbg["hp2"].ap(), hp2[:])
                sy.dma_start(dbg["cb2"].ap(), colbank[:])
            if DEBUG and li == 3:
                for c in range(3):
                    sy.dma_start(dbg[f"hp3_{c}"].ap(), hp3[c][:])
                    sy.dma_start(dbg[f"up3_{c}"].ap(), upool[c][:])
                    sy.dma_start(dbg[f"v3_{c}"].ap(), V[c][:])
                sy.dma_start(dbg["cb3"].ap(), colbank[:])
                sy.dma_start(dbg["sb3"].ap(), dbg_sb[:])
                for pt in range(8):
                    sy.dma_start(dbg[f"u3_{pt}"].ap(), U[pt][:])
                cntpack = work.tile([128, 8], F32, tag="cntpack",
                                    name="cntpack", bufs=1)
                for pt in range(8):
                    v.tensor_copy(cntpack[:, pt:pt + 1],
                                  CNT32[pt // 2][pt % 2][:])
                sy.dma_start(dbg["cnt"].ap(), cntpack[:])
            if li == 1:
                gp.collective_compute("AllGather", OP.bypass,
                                      replica_groups=rg,
                                      ins=[ag1_in.ap()],
                                      outs=[ag1_out.ap()])
            elif li == 2:
                gp.collective_compute("AllGather", OP.bypass,
                                      replica_groups=rg,
                                      ins=[ag2_in.ap()],
                                      outs=[ag2_out.ap()])

        # ---------------- L4: 1x1 conv over local 1104 channels ----------
        if STAGE < 5:
            zz = work.tile([128, PTS], F32, tag="hwork", name="zz")
            v.memset(zz[:], 0.0)
            dstz = out_d.ap().rearrange("b t c n -> c (b t) n")
            for half in range(2):
                sy.dma_start(dstz[half * 128:(half + 1) * 128],
                             zz[:].rearrange("p (f n) -> p f n", n=N))
        pieces = [(hp0, 48, False), (tp0, 48, False),
                  (hp1, 24, False), (tp1r, 24, True),
                  (hp2, 96, False), (tp2r, 96, True)]
        for ct in range(3):
            pieces.append((hp3[ct], 128, False))
        for ct in range(3):
            pieces.append((tp3r[ct], 128, True))

        for ch in range(2 if STAGE >= 5 else 0):
            pys0 = pa.tile([128, 512], F32, tag="pa", name="pys0")
            pys1 = pa.tile([128, 512], F32, tag="pa", name="pys1")
            ro = 0
            for bi, (piece, rows, rep) in enumerate(pieces):
                wb = work.tile([128, 256], F16, tag="w4s", name="wb",
                               bufs=2)[:rows, :]
                sy.dma_start(wb[:], io["w4t"].ap()[ro:ro + rows])
                rhs = (piece[0:rows, 0:512] if rep
                       else piece[0:rows, ch * 512:(ch + 1) * 512])
                te.matmul(pys0[:], wb[:, 0:128], rhs, start=(bi == 0),
                          stop=(bi == len(pieces) - 1))
                te.matmul(pys1[:], wb[:, 128:256], rhs, start=(bi == 0),
                          stop=(bi == len(pieces) - 1))
                ro += rows
            yp = work.tile([128, PTS], F32, tag="hwork", name="yp")
            sc.copy(yp[:, 0:512], pys0[:])
            sc.copy(yp[:, 512:1024], pys1[:])
            sy.dma_start(ar_in.ap()[0:128, ch * 512:(ch + 1) * 512],
                         yp[:, 0:512])
            sy.dma_start(ar_in.ap()[128:256, ch * 512:(ch + 1) * 512],
                         yp[:, 512:1024])
        if STAGE >= 5:
            gp.collective_compute("AllReduce", OP.add, replica_groups=rg,
                                  ins=[ar_in.ap()], outs=[ar_out.ap()])

        # ---------------- final BN + lrelu + store ----------------
        for half in range(2 if STAGE >= 5 else 0):
            yf = work.tile([128, PTS], F32, tag="hwork", name="yf")
            sy.dma_start(yf[:], ar_out.ap()[half * 128:(half + 1) * 128])
            scy = work.tile([128, QK], F16, tag="E2", name="scy")[:, :PTS]
            sc.activation(scy[:], yf[:], AF.Copy,
                          accum_out=colbank[:, 0:1])
            sc.activation(scy[:], yf[:], AF.Square,
                          accum_out=colbank[:, 1:2])
            cb = colbank
            col = lambda i: cb[:, i:i + 1]
            v.tensor_scalar(col(2), col(0), 1.0 / PTS, None, OP.mult)
            v.tensor_scalar(col(3), col(1), 1.0 / PTS, None, OP.mult)
            v.tensor_tensor(col(4), col(2), col(2), OP.mult)
            v.tensor_tensor(col(4), col(3), col(4), OP.subtract)
            std = work.tile([128, 1], F32, tag="stdcol", name="ystd")
            sc.activation(std[:], col(4), AF.Sqrt, bias=eps_col[:])
            inv = work.tile([128, 1], F32, tag="invcol", name="yinv")
            v.reciprocal(inv[:], std[:])
            gcol = work.tile([128, 1], F32, tag="ygcol", name="ygcol")
            sy.dma_start(gcol[:], io["g4"].ap()[half * 128:(half + 1) * 128])
            bcol = work.tile([128, 1], F32, tag="ybcol", name="ybcol")
            sy.dma_start(bcol[:], io["b4"].ap()[half * 128:(half + 1) * 128])
            scal = work.tile([128, 1], F32, tag="scalcol", name="yscal")
            v.tensor_tensor(scal[:], inv[:], gcol[:], OP.mult)
            bias2 = work.tile([128, 1], F32, tag="biascol", name="ybias")
            v.tensor_tensor(bias2[:], col(2), scal[:], OP.mult)
            v.tensor_tensor(bias2[:], bcol[:], bias2[:], OP.subtract)
            yo = work.tile([128, PTS], F32, tag="hwork", name="yo")
            sc.activation(yo[:], yf[:], AF.Prelu, bias=bias2[:],
                          scale=scal[:], alpha=0.2)
            dst = out_d.ap().rearrange("b t c n -> c (b t) n")
            sy.dma_start(dst[half * 128:(half + 1) * 128],
                         yo[:].rearrange("p (f n) -> p f n", n=N))

        for p in (pa, p1, work, pers):
            p.release()

    nc.compile()
    return nc


def _perm_for(C8):
    C = C8 * NCORE
    out = []
    for r in range(NCORE):
        out += list(range(r * C8, (r + 1) * C8))
        out += list(range(C + r * C8, C + (r + 1) * C8))
    return np.array(out)


def _prep_inputs(inputs):
    x = np.asarray(inputs["x"], np.float32)
    xcols = np.ascontiguousarray(x.reshape(PTS, 3).T)
    base = {
        "xf32": xcols,
        "x16": xcols.astype(np.float16),
        "iota": np.arange(256, dtype=np.float32).reshape(256, 1),
        "negones": np.full((3, 128), -1.0, np.float32),
        "g4": np.asarray(inputs["g4"], np.float32).reshape(256, 1),
        "b4": np.asarray(inputs["b4"], np.float32).reshape(256, 1),
    }
    perm = np.arange(3)
    in_maps = [dict(base) for _ in range(NCORE)]
    for li in range(4):
        C = COUT[li]
        F = FIN[li]
        W = np.asarray(inputs[f"w{li}"], np.float32)
        Wa_full = W[:, :F].copy()
        Wd_full = (W[:, F:] - W[:, :F]).copy()
        if li > 0:
            # tp rows of the input comb hold (h_f0 + h_f1); fold the 0.5
            Cp = COUT[li - 1]
            Wa_full[:, Cp:2 * Cp] *= 0.5
            Wd_full[:, Cp:2 * Cp] *= 0.5
        Wa_full = Wa_full[:, perm]
        Wd_full = Wd_full[:, perm]
        g = np.asarray(inputs[f"g{li}"], np.float32)
        b = np.asarray(inputs[f"b{li}"], np.float32)
        for r in range(NCORE):
            if li == 0:
                rows = slice(0, 48)
            else:
                rows = slice(r * SL[li], (r + 1) * SL[li])
            in_maps[r][f"wa{li}"] = np.ascontiguousarray(
                Wa_full[rows].T).astype(np.float16)
            in_maps[r][f"wd{li}"] = np.ascontiguousarray(
                Wd_full[rows].T).astype(np.float16)
            in_maps[r][f"g{li}"] = g[rows].reshape(-1, 1).copy()
            in_maps[r][f"b{li}"] = b[rows].reshape(-1, 1).copy()
        perm = np.arange(2 * C) if li == 0 else _perm_for(C // NCORE)
    w4 = np.asarray(inputs["w4"], np.float32)
    for r in range(NCORE):
        cols = []
        cols.append((np.arange(0, 48), 1 / 8))          # hp0 (replicated)
        cols.append((np.arange(48, 96), 1 / 16))        # tp0 (sum stored)
        cols.append((96 + np.arange(r * 24, (r + 1) * 24), 1.0))
        cols.append((288 + np.arange(r * 24, (r + 1) * 24), 0.5))
        cols.append((480 + np.arange(r * 96, (r + 1) * 96), 1.0))
        cols.append((1248 + np.arange(r * 96, (r + 1) * 96), 0.5))
        cols.append((2016 + np.arange(r * 384, (r + 1) * 384), 1.0))
        cols.append((5088 + np.arange(r * 384, (r + 1) * 384), 0.5))
        blocks = [np.ascontiguousarray(w4[:, c].T) * s for c, s in cols]
        in_maps[r]["w4t"] = np.concatenate(blocks, axis=0).astype(np.float16)
    return in_maps


_NC_CACHE = []


def kernel(**inputs):
    if not _NC_CACHE:
        _NC_CACHE.append(build_nc())
    nc = _NC_CACHE[0]
    in_maps = _prep_inputs(inputs)
    res = run_bass_kernel_spmd(nc, in_maps, list(range(NCORE)))
    return np.asarray(res.results[0]["out"])
